# revision 53
# baseline (speedup 1.0000x reference)
"""Trainium2 Bass kernel for nn_AsymmetricProjectedLinear (8 NeuronCores).

Reference computes out = x @ W_large^T with
    W_large = (A_out @ B_out) @ W_small @ (A_in @ B_in)^T    [4096, 4096]

W_large is never materialized. Factored (~2.3 GFLOP vs ~137 naive):
    H  = B_in @ W_small^T                       [64, 1024]
    M  = H @ B_out^T                            [64, 64]
    per 256-token block: u1 = x @ A_in; t2 = u1 @ M; out = t2 @ A_out^T

Sharding: tokens (B*S = 4096) split 512/core across 8 cores; weights
replicated (a 16KB AllReduce for M costs ~50us wall on this runtime, so
every core redundantly computes M from the full W_small). Host work is
layout-only (transpose/pack/slice/dtype-cast); all FLOPs on-device.
Compute dtype bf16 (rel err 5.5e-3 vs the f32 reference; this problem
family's gate is 2e-2).

The kernel is wire-bound: ~11.6MB of HBM traffic per core at an
observed ~360-400GB/s across both HWDGE rings, plus ~7us of fixed
runtime preamble and ~3us of completion/barrier tail. Measured
~49.5-55us HW exec (core-to-core HBM contention gives +-3us run to
run); the two-stage baseline this replaced ran ~59-67us.

Hard-won structure notes (from perfetto traces of ~15 versions):
  - The Tile scheduler orders each engine's static queue by a
    cost-model simulation that badly mispredicts DMA arrivals, and
    engines dispatch in-order, so a mis-ordered queue head-of-line
    blocks ready work (+5..20us in early versions). Every engine queue
    is pinned to emission order with sync=False dep edges: emission
    order here IS the schedule.
  - Ring order = arrival order (HWDGE rings drain FIFO). Wire order:
    small weights, W^T (one DMA per ring; each ring DMA carries ~1us
    of dead time, so fewer/bigger transfers), x(b0), a_out, x(b1),
    with out DMAs chained behind on both rings.
  - Sync/Scalar sequencers issue their ring's DMAs and stall on ring
    backpressure, so the Scalar ENGINE cannot run compute until its
    ring's in-stream issues drain (~22us); prework drains that must
    run earlier go on Vector.
  - The PE clock-throttles ~2x when idle/sparse; the prework tail
    (transposes + M) is emitted twice (idempotent) to stay dense.
  - stage1 matmuls read their moving operand at 109ns from
    4KB/partition x tiles but 213ns from 8KB/partition tiles
    (measured repeatedly), so x streams as 4x [128, 2048] pieces per
    block.
  - Interleaved matmul accumulation groups inside ONE psum tile
    compute garbage on HW (verified in isolation); groups must be
    sequential per tile or live in separate tiles (H uses two tiles).
  - The PE dual-pumps adjacent matmuls whose psum tiles sit in
    opposite partition halves (row- or col-offset tile_position): the
    second of each pair costs ~5ns. Used for: t2 written to BOTH
    psum halves by a col-offset pair (replaces an SBUF dup DMA), and
    stage5 K=64 row-offset pairs against the two stacked halves of
    A_out^T (a_out2 packs A_out^T cols 0:2048 in partitions 0-63 and
    cols 2048:4096 in 64-127).
  - stage1 is one M=64/N=256 chain per block (109ns/MM warm,
    LDWEIGHTS hidden; N=128 chains hit a ~107ns LDW floor and double
    PE time).
  - PSUM->SBUF drains run ~95G elem/s (PSUM source caps DVE at 1x),
    so the out tiles' 2.1M elems cost ~10.5us split across
    Vector+Scalar: the back half is drain-paced. stage1(b1) chunks
    interleave into the drain-gated stage5(b0) pair slots (pairs 2-5,
    close/t2(b1) at pairs 5-6) so stage5(b1) starts right behind.
  - Out tiles are [128, 2048] (524KB DMAs, 4 bufs/tag: no recycling;
    merging to [128, 4096] single-DMA tiles measured worse - the DMA
    then waits on all 8 drains).
"""

import numpy as np

import concourse.bass as bass
import concourse.mybir as mybir
import concourse.tile as tile
from concourse import bacc
from concourse.bass_utils import run_bass_kernel_spmd
from concourse.tile_rust import add_dep_helper

N_CORES = 8
Bsz, S, D = 2, 2048, 4096
TOK = Bsz * S          # 4096 tokens
T = TOK // N_CORES     # 512 tokens per core
TB = 256               # tokens per pipeline block
NBLK = T // TB         # 2 blocks
RANK = 64
DS = 1024              # d_small

F32 = mybir.dt.float32
BF16 = mybir.dt.bfloat16

_nc_cache = {}


def build():
    if "nc" in _nc_cache:
        return _nc_cache["nc"]
    nc = bacc.Bacc("TRN2", target_bir_lowering=False, debug=False,
                   num_devices=N_CORES)

    # x_p: per block, 32 d-tiles of [128, TB] packed -> [128, 32*TB]
    x_p = nc.dram_tensor("x_p", [NBLK, 128, 32 * TB], BF16, kind="ExternalInput")
    b_outT_p = nc.dram_tensor("b_outT_p", [128, 8 * RANK], BF16,
                              kind="ExternalInput")
    b_inT_p = nc.dram_tensor("b_inT_p", [128, 8 * RANK], BF16,
                             kind="ExternalInput")
    a_in_p = nc.dram_tensor("a_in_p", [128, 32 * RANK], BF16, kind="ExternalInput")
    # A_out^T stacked: parts 0-63 = cols 0:2048, parts 64-127 = cols 2048:4096
    a_out2 = nc.dram_tensor("a_out2", [128, 2048], BF16, kind="ExternalInput")
    # W_small^T packed d_in-major: chunk j = d_in rows [j*128,(j+1)*128)
    wT_p = nc.dram_tensor("wT_p", [128, 8 * DS], BF16, kind="ExternalInput")
    ident = nc.dram_tensor("ident", [RANK, RANK], BF16, kind="ExternalInput")
    out = nc.dram_tensor("out", [T, D], BF16, kind="ExternalOutput")

    # Per-engine emission-order chains (sync=False: ordering only).
    last = {}

    def chain(key, bi):
        if key in last:
            add_dep_helper(bi.ins, last[key].ins, sync=False,
                           reason="emission-order schedule")
        last[key] = bi
        return bi

    with tile.TileContext(nc) as tc:
        with (
            tc.tile_pool(name="const", bufs=1) as cpool,
            tc.tile_pool(name="xin", bufs=2) as xpool,
            tc.tile_pool(name="outp", bufs=4) as opool,
            tc.tile_pool(name="interm", bufs=2) as ipool,
            tc.tile_pool(name="ps_pre", bufs=2, space="PSUM") as ps_pre,
            tc.tile_pool(name="ps_s1", bufs=2, space="PSUM") as ps_s1,
            tc.tile_pool(name="ps_o", bufs=4, space="PSUM") as ps_o,
        ):
            # ---- input streams, byte-balanced across BOTH HWDGE rings --
            # Ring order = arrival order: weights needed by prework first,
            # then a_in, x(b0), a_out (needed only at stage5 ~24us),
            # x(b1); out DMAs chained behind.
            b_outT_s = cpool.tile([128, 8 * RANK], BF16)
            b_inT_s = cpool.tile([128, 8 * RANK], BF16)
            a_in_s = cpool.tile([128, 32 * RANK], BF16)
            a_out_s = cpool.tile([128, 2048], BF16)
            ident_s = cpool.tile([RANK, RANK], BF16)
            w_tiles = [None] * 8
            x_tiles = [[None] * 4 for _ in range(NBLK)]

            chain("A", nc.sync.dma_start(out=b_inT_s[:, :], in_=b_inT_p.ap()))
            chain("B", nc.scalar.dma_start(out=ident_s[:, :], in_=ident.ap()))
            chain("B", nc.scalar.dma_start(out=b_outT_s[:, :], in_=b_outT_p.ap()))
            wA_s = cpool.tile([128, 4 * DS], BF16)
            wB_s = cpool.tile([128, 4 * DS], BF16)
            chain("A", nc.sync.dma_start(out=wA_s[:, :],
                                         in_=wT_p.ap()[:, 0:4 * DS]))
            chain("B", nc.scalar.dma_start(out=wB_s[:, :],
                                           in_=wT_p.ap()[:, 4 * DS:8 * DS]))
            for j in range(8):
                ws = wA_s if j < 4 else wB_s
                w_tiles[j] = ws[:, (j % 4) * DS:(j % 4 + 1) * DS]
            chain("A", nc.sync.dma_start(out=a_in_s[:, 0:1024],
                                         in_=a_in_p.ap()[:, 0:1024]))
            chain("B", nc.scalar.dma_start(out=a_in_s[:, 1024:2048],
                                           in_=a_in_p.ap()[:, 1024:2048]))

            def x_dma(b):
                # 4 pieces of [128, 2048] (4KB/partition): stage1 matmuls
                # read rhs from 4KB/part tiles at 109ns vs 213ns from
                # 8KB/part tiles (measured across many versions).
                for p in range(4):
                    xt = xpool.tile([128, 8 * TB], BF16, tag=f"x{p}")
                    eng, key = (nc.sync, "A") if p < 2 else (nc.scalar, "B")
                    chain(key, eng.dma_start(
                        out=xt[:, :],
                        in_=x_p.ap()[b, :, p * 8 * TB:(p + 1) * 8 * TB],
                    ))
                    x_tiles[b][p] = xt

            x_dma(0)
            chain("A", nc.sync.dma_start(out=a_out_s[:, 0:1024],
                                         in_=a_out2.ap()[:, 0:1024]))
            chain("B", nc.scalar.dma_start(out=a_out_s[:, 1024:2048],
                                           in_=a_out2.ap()[:, 1024:2048]))
            x_dma(1)

            # ---- prework: H -> H^T -> M ----
            # H = B_in @ W_small^T [64, 1024], accumulated over d_in
            # chunks j as they land. Two psum tiles, one group each.
            h_ps = [ps_pre.tile([RANK, 512], F32, tag="pre", name=f"h_ps{hh}")
                    for hh in range(2)]
            for j in range(8):
                for hh in range(2):
                    chain("T", nc.tensor.matmul(
                        h_ps[hh][:, :],
                        b_inT_s[:, j * RANK:(j + 1) * RANK],
                        w_tiles[j][:, hh * 512:(hh + 1) * 512],
                        start=(j == 0), stop=(j == 7),
                    ))
            h_s = cpool.tile([RANK, DS], BF16)
            chain("V", nc.vector.tensor_copy(h_s[:, 0:512], h_ps[0][:, :]))
            chain("V", nc.vector.tensor_copy(h_s[:, 512:1024], h_ps[1][:, :]))

            # ---- prework tail: H^T and M (emitted later, between
            # stage1(b0) and stage2(b0) — stage1 needs only a_in+x and
            # runs first so the PE stays dense while x(b0) lands) ----
            def prework_tail():
                hT_s = cpool.tile([128, 8 * RANK], BF16)
                for t in range(8):
                    ht_ps = ps_pre.tile([128, RANK], BF16, tag="pre")
                    chain("T", nc.tensor.transpose(
                        ht_ps[:, :], h_s[:, t * 128:(t + 1) * 128],
                        ident_s[:, :]))
                    chain("V", nc.vector.tensor_copy(
                        hT_s[:, t * RANK:(t + 1) * RANK], ht_ps[:, :]))
                # M = H @ B_out^T [r_in, r_out], accumulated over d_out
                m_s = cpool.tile([RANK, RANK], BF16)
                m_ps = ps_pre.tile([RANK, RANK], F32, tag="pre")
                for t in range(8):
                    chain("T", nc.tensor.matmul(
                        m_ps[:, :],
                        hT_s[:, t * RANK:(t + 1) * RANK],
                        b_outT_s[:, t * RANK:(t + 1) * RANK],
                        start=(t == 0), stop=(t == 7),
                    ))
                chain("V", nc.vector.tensor_copy(m_s[:, :], m_ps[:, :]))
                return m_s

            # ---- per token block ----
            u1_psb = {}

            def stage1_chunks(b, mlo, mhi):
                if b not in u1_psb:
                    u1_psb[b] = ps_s1.tile([RANK, TB], F32, tag="s1",
                                           name=f"u1_ps{b}")
                u1_ps = u1_psb[b]
                for m in range(mlo, mhi):
                    xt = x_tiles[b][m // 8]
                    col = (m % 8) * TB
                    chain("T", nc.tensor.matmul(
                        u1_ps[:, :],
                        a_in_s[:, m * RANK:(m + 1) * RANK],
                        xt[:, col:col + TB],
                        start=(m == 0), stop=(m == 31),
                    ))

            def stage1_close(b):
                u1_s = ipool.tile([RANK, TB], BF16, tag="u1")
                chain("V", nc.vector.tensor_copy(u1_s[:, :], u1_psb[b][:, :]))
                return u1_s

            def stage2(b, u1_s, m_s):
                # t2 = (u1 @ M)^T, written by the PE to BOTH partition
                # halves (col tile offset pair dual-pumps, ~5ns extra).
                # Emitted per token-half so stage5's s=0 pairs start as
                # soon as the first half is drained.
                t2_ps = ps_s1.tile([128, TB], F32, tag="s1")
                t2b = ipool.tile([128, TB], BF16, tag="t2")
                for s in range(2):
                    cols = slice(s * 128, (s + 1) * 128)
                    for ch in range(2):
                        chain("T", nc.tensor.matmul(
                            t2_ps[ch * RANK:(ch + 1) * RANK, cols],
                            m_s[:, :], u1_s[:, cols], start=True, stop=True,
                        ))
                    chain("S", nc.scalar.copy(t2b[:, cols], t2_ps[:, cols]))
                return t2b

            def stage5_pair(b, t2b, s, n, o_ts):
                # pair (s, n): po0 = tokens s-slice x out cols n*512
                # (lo half), po1 = same tokens x cols 2048+n*512
                po0 = ps_o.tile([128, 512], F32, tag="po")
                po1 = ps_o.tile([128, 512], F32, tag="po")
                chain("T", nc.tensor.matmul(
                    po0[:, :], t2b[0:RANK, s * 128:(s + 1) * 128],
                    a_out_s[0:RANK, n * 512:(n + 1) * 512],
                    start=True, stop=True,
                ))
                chain("T", nc.tensor.matmul(
                    po1[:, :], t2b[RANK:128, s * 128:(s + 1) * 128],
                    a_out_s[RANK:128, n * 512:(n + 1) * 512],
                    start=True, stop=True,
                ))
                chain("V", nc.vector.tensor_copy(
                    o_ts[0][:, n * 512:(n + 1) * 512], po0[:, :]))
                chain("S", nc.scalar.copy(
                    o_ts[1][:, n * 512:(n + 1) * 512], po1[:, :]))

            def out_dma(b, s, o_ts):
                r0 = b * TB + s * 128
                ek = [(nc.sync, "A"), (nc.scalar, "B")]
                if s == 1:
                    ek = ek[::-1]
                for cg in range(2):
                    e, key = ek[cg]
                    chain(key, e.dma_start(
                        out=out.ap()[r0:r0 + 128, cg * 2048:(cg + 1) * 2048],
                        in_=o_ts[cg][:, :]))

            # Emission = schedule. The prework tail (transposes + M) and
            # stage1(b0)'s first x piece become ready at about the same
            # time (~21us), so their PE work is interleaved to keep the
            # PE dense (it clock-throttles when idle). stage5(b0) pairs
            # are drain-gated, so stage1(b1) chunks interleave into the
            # PE idle slots once x(b1) pieces have landed.
            m_s = prework_tail()
            stage1_chunks(0, 0, 32)
            u1_b0 = stage1_close(0)
            t2_b0 = stage2(0, u1_b0, m_s)
            o_b0 = [[opool.tile([128, 2048], BF16, tag=f"o{s}{cg}", name=f"o0_{s}{cg}")
                     for cg in range(2)] for s in range(2)]
            pair_i = 0
            u1_b1 = t2_b1 = None
            for s in range(2):
                for n in range(4):
                    stage5_pair(0, t2_b0, s, n, o_b0[s])
                    if 2 <= pair_i <= 5:
                        mlo = (pair_i - 2) * 8
                        stage1_chunks(1, mlo, mlo + 8)
                    if pair_i == 5:
                        u1_b1 = stage1_close(1)
                    if pair_i == 6:
                        t2_b1 = stage2(1, u1_b1, m_s)
                    pair_i += 1
                out_dma(0, s, o_b0[s])
            o_b1 = [[opool.tile([128, 2048], BF16, tag=f"o{s}{cg}", name=f"o1_{s}{cg}")
                     for cg in range(2)] for s in range(2)]
            for s in range(2):
                for n in range(4):
                    stage5_pair(1, t2_b1, s, n, o_b1[s])
                out_dma(1, s, o_b1[s])

    nc.compile()
    _nc_cache["nc"] = nc
    return nc


def _prep_in_maps(x, W_small, A_out, B_out, A_in, B_in):
    import ml_dtypes
    f = ml_dtypes.bfloat16
    x2 = np.asarray(x, dtype=f).reshape(TOK, D)
    a_in_p = np.ascontiguousarray(
        np.asarray(A_in, f).reshape(32, 128, RANK).transpose(1, 0, 2)
    ).reshape(128, 32 * RANK)
    a_outT = np.asarray(A_out, f).T            # [64, 4096]
    a_out2 = np.ascontiguousarray(
        np.concatenate([a_outT[:, :2048], a_outT[:, 2048:]], axis=0))
    b_inT_p = np.ascontiguousarray(
        np.asarray(B_in, f).T.reshape(8, 128, RANK).transpose(1, 0, 2)
    ).reshape(128, 8 * RANK)
    b_outT_p = np.ascontiguousarray(
        np.asarray(B_out, f).T.reshape(8, 128, RANK).transpose(1, 0, 2)
    ).reshape(128, 8 * RANK)
    wT_p = np.ascontiguousarray(
        np.asarray(W_small, f).T.reshape(8, 128, DS).transpose(1, 0, 2)
    ).reshape(128, 8 * DS)
    ident = np.eye(RANK, dtype=f)
    shared = {
        "b_outT_p": b_outT_p, "b_inT_p": b_inT_p, "a_in_p": a_in_p,
        "a_out2": a_out2, "wT_p": wT_p, "ident": ident,
    }
    in_maps = []
    for c in range(N_CORES):
        xs = x2[c * T:(c + 1) * T, :]            # [T, 4096]
        xp = np.ascontiguousarray(
            xs.T                                  # [4096, T]
            .reshape(32, 128, NBLK, TB)           # d-tile, p, blk, t
            .transpose(2, 1, 0, 3)                # blk, p, d-tile, t
        ).reshape(NBLK, 128, 32 * TB)
        in_maps.append({"x_p": xp, **shared})
    return in_maps


def _run(inputs, trace=False):
    nc = build()
    in_maps = _prep_in_maps(**inputs)
    res = run_bass_kernel_spmd(
        nc, in_maps, core_ids=list(range(N_CORES)), trace=trace
    )
    out = np.concatenate(
        [np.asarray(res.results[c]["out"], dtype=np.float32)
         for c in range(N_CORES)], axis=0
    ).reshape(Bsz, S, D)
    return out, res


def kernel(**inputs) -> np.ndarray:
    out, _ = _run(inputs, trace=False)
    return out
